# revision 29
# baseline (speedup 1.0000x reference)
"""Trainium2 Bass kernel for nn_Attention_17738214932808.

Computation (per batch b):
    mids   = q @ W.T                               [B, D]
    scores = tanh(k . mids + bias)                 [B, T]
    attn   = softmax-with-mask:  e = exp(scores - max) * m ; attn = e / sum(e)
Since tanh is bounded in (-1, 1), the max-subtraction is a mathematical no-op
for the final ratio (exp(s-c)/sum m exp(s-c) is invariant in c), so we compute
e = exp(scores) * m directly; fp32-rounding-level difference only.

Sharding: data-parallel over batch, 8 batches per NeuronCore x 8 cores.

Layout trick: each SBUF partition loads a CONTIGUOUS 16KB run of k (16 t-rows),
so k's DMA runs at ~HBM peak. The resulting score-column permutation is fixed
up by pre-permuting m and inverse-permuting the output on the host (pure input
marshalling; all FLOPs happen on-device).

Dot-product engine: a single custom DVE op (PRODUCT_CUMSUM_ANT) computes the
running prefix sum of k*mids over each partition's whole 4096-element stream
in ONE instruction per [128, 16, 256] tile. The 16 per-partition dot products
are then recovered as differences of the 16 page-end cumsum values (2 tiny DVE
ops per tile). This replaces ~300 per-subtile multiply/reduce instructions
(DVE ~118us + ACT ~116us busy in the previous version) with 16 full-rate
streaming instructions (~70us DVE busy), dropping below the ~94us/core HBM
floor for the 33.5MB k-slice. Numerics: cumsum cancellation error is ~1e-4
absolute on dots of magnitude ~1e2 (fp32 state), far inside tolerance.
"""

import os

import numpy as np

import concourse.bass as bass
import concourse.tile as tile
from concourse import bacc, mybir
from concourse.bass_utils import run_bass_kernel_spmd
from concourse.masks import make_identity

F32 = mybir.dt.float32
AF = mybir.ActivationFunctionType
ALU = mybir.AluOpType

B, T, D = 64, 4096, 256
NCORES = 8
BL = B // NCORES          # batches per core = 8
H = 2                     # halves of T per batch (macro tiles)
TT = 16                   # t-pages per macro  (T = H * 128 * TT)
P = 128

LAST_RESULTS = None       # BassKernelResults of the most recent run (for test.py)


# ---------------------------------------------------------------------------
# Custom DVE op: out[p, :] = cumsum(in0[p, :] * in1[p, :]) along the free dim.
# Registered once at import; sha pins are computed locally so compile()'s
# golden check always matches.
# ---------------------------------------------------------------------------
def _register_product_cumsum():
    import concourse.dve_ops as dve_ops
    from concourse.dve_spec import Spec, Src0, Src1, scan, AluOp, lower
    from concourse.dve_uop import DveOpSpec

    name = "PRODUCT_CUMSUM_ANT"
    if name in dve_ops._SUB_OPCODE_FOR_NAME:
        return next(op for op in dve_ops.OPS if op.name == name)

    def _ref(in0, in1, s0, s1, imm2):
        p = (np.asarray(in0, np.float32) * np.asarray(in1, np.float32))
        flat = p.reshape(p.shape[0], -1)
        return np.cumsum(flat, axis=1, dtype=np.float32).reshape(p.shape)

    spec = Spec(body=scan(AluOp.ADD, Src0 * Src1), reference=_ref)
    shas = {}
    for ver in ("v3", "v4"):
        shas[ver] = DveOpSpec(
            name=name, uops=lower(spec, ver=ver), rd1_en=True
        ).sha(ver)
    op = dve_ops.DveOp(name, spec, subdim=False, uops_sha=shas)
    row = dve_ops._CUSTOM_DVE_ROW_BASE + len(dve_ops.OPS)
    assert row < 0x20, "custom DVE opcode rows exhausted"
    dve_ops.OPS.append(op)
    dve_ops._SUB_OPCODE_FOR_NAME[name] = row
    dve_ops.CUSTOM_DVE_SPECS[name] = spec
    return op


PRODUCT_CUMSUM = _register_product_cumsum()


def _broadcast_row(ap, nparts):
    """[1, N] AP -> [nparts, N] AP with partition step 0."""
    try:
        return ap.to_broadcast([nparts] + list(ap.shape[1:]))
    except Exception:
        return bass.AP(
            tensor=ap.tensor,
            offset=ap.offset,
            ap=[[0, nparts]] + [list(d) for d in ap.ap[1:]],
        )


def _build_kernel(ctx, tc, outs, ins):
    nc = tc.nc
    q, k, mp, W, bias = ins["q"], ins["k"], ins["mp"], ins["W"], ins["bias"]
    out = outs["out"]

    consts = ctx.enter_context(tc.tile_pool(name="consts", bufs=1))
    setup = ctx.enter_context(tc.tile_pool(name="setup", bufs=2))
    kpool = ctx.enter_context(tc.tile_pool(name="kpool", bufs=5))
    cumpool = ctx.enter_context(tc.tile_pool(name="cumpool", bufs=2))
    mtpool = ctx.enter_context(tc.tile_pool(name="mtpool", bufs=4))
    scpool = ctx.enter_context(tc.tile_pool(name="scores", bufs=3))
    epool = ctx.enter_context(tc.tile_pool(name="epil", bufs=2))
    ps_misc = ctx.enter_context(tc.tile_pool(name="ps_misc", bufs=4, space="PSUM"))
    ps_e = ctx.enter_context(tc.tile_pool(name="ps_e", bufs=2, space="PSUM"))

    # ---------------- Phase 0: constants + mids = q @ W.T ----------------
    # W/q go on the scalar HWDGE queue, whose DGE spins up ~2us before the
    # sync queue's k stream starts — they transfer at full stripe rate in
    # that window, so the mb chain and k tile 1 race evenly (~14us).
    w_sb = setup.tile([P, 2, D], F32, tag="w")
    nc.scalar.dma_start(out=w_sb[:], in_=W.ap().rearrange("(dc p) e -> p dc e", p=P))
    q_sb = setup.tile([BL, D], F32, tag="q")
    nc.scalar.dma_start(out=q_sb[:], in_=q.ap())

    ident = consts.tile([P, P], F32)
    make_identity(nc, ident)

    bias_col = consts.tile([P, 1], F32)
    nc.gpsimd.dma_start(out=bias_col[:], in_=_broadcast_row(bias.ap(), P))

    # ones block-diagonal [64, 2]: blk[p, g] = 1 iff p//32 == g
    blk = consts.tile([64, 2], F32)
    nc.gpsimd.memset(blk[:], 1.0)
    nc.gpsimd.affine_select(   # keep where p - 32g >= 0
        out=blk[:], in_=blk[:], compare_op=ALU.is_ge, fill=0.0,
        base=0, pattern=[[-32, 2]], channel_multiplier=1,
    )
    nc.gpsimd.affine_select(   # keep where 31 - p + 32g >= 0  (i.e. p - 32g <= 31)
        out=blk[:], in_=blk[:], compare_op=ALU.is_ge, fill=0.0,
        base=31, pattern=[[32, 2]], channel_multiplier=-1,
    )
    # selector [2, 64]: sel[g, x] = 1 iff x//32 == g
    sel = consts.tile([2, 64], F32)
    nc.gpsimd.memset(sel[:], 0.0)
    nc.gpsimd.affine_select(   # iota = g - (x//32); equal -> fill... keep where != 0
        out=sel.rearrange("p (g x) -> p g x", g=2),
        in_=sel.rearrange("p (g x) -> p g x", g=2),
        compare_op=ALU.not_equal, fill=1.0,
        base=0, pattern=[[-1, 2], [0, 32]], channel_multiplier=1,
    )
    # W^T chunks via PE transposes, all into ONE psum bank -> one DVE copy.
    # wt[p=e_local, i=(dc, ec), d_local]
    pst_w = ps_misc.tile([P, 4, P], F32, tag="mix")
    for dc in range(2):
        for ec in range(2):
            nc.tensor.transpose(
                pst_w[:, dc * 2 + ec, :], w_sb[:, dc, ec * P:(ec + 1) * P], ident[:]
            )
    wt = setup.tile([P, 4, P], F32, tag="wt")
    nc.vector.tensor_copy(wt[:], pst_w[:])
    # q^T chunks: qt[p=e_local, ec, b]
    pst_q = ps_misc.tile([P, 2, BL], F32, tag="mix")
    for ec in range(2):
        nc.tensor.transpose(
            pst_q[:, ec, :], q_sb[:, ec * P:(ec + 1) * P], ident[0:BL, 0:BL]
        )
    qt = setup.tile([P, 2, BL], F32, tag="qt")
    nc.vector.tensor_copy(qt[:], pst_q[:])

    # mb[p', d] = mids[b, d] = sum_e W[d, e] q[b, e], computed directly on
    # the PE for all 128 partitions at once: lhsT = qt column b broadcast
    # across the out-partition dim, rhs = the matching W^T chunk. The scans
    # read it straight from PSUM. Pair g+1's pair of mb tiles is emitted
    # right after pair g's scans (before pair g's epilogue PE work) so the
    # next pair's first scan never stalls behind the epilogue.
    ps_mb = ctx.enter_context(tc.tile_pool(name="ps_mb", bufs=2, space="PSUM"))

    def emit_mb(g):
        mb = ps_mb.tile([P, 2, D], F32)
        for b_local in range(2):
            b = g * 2 + b_local
            for dc in range(2):
                for ec in range(2):
                    nc.tensor.matmul(
                        mb[:, b_local, dc * P:(dc + 1) * P],
                        lhsT=qt[:, ec, b:b + 1].broadcast_to([P, P]),
                        rhs=wt[:, dc * 2 + ec, :],
                        start=(ec == 0), stop=(ec == 1),
                    )
        return mb

    mb_ps = emit_mb(0)

    # ---------------- Phase 1: main loop + epilogue per batch-pair ----------------
    for g in range(BL // 2):                 # 4 pairs
        scores = scpool.tile([P, 64], F32)   # col = b_local*32 + h*16 + s
        # prefetch this pair's mask tile (needed only in the epilogue)
        mt = mtpool.tile([64, P], F32, tag="mt")
        nc.gpsimd.dma_start(
            out=mt[:],
            in_=mp.ap()[g * 2:(g + 1) * 2].rearrange("b c p -> (b c) p"),
        )

        for b_local in range(2):
            b = g * 2 + b_local
            for h in range(2):
                kt = kpool.tile([P, TT, D], F32, tag="ktile")
                nc.sync.dma_start(
                    out=kt[:],
                    in_=k.ap()[b, h * 2048:(h + 1) * 2048, :].rearrange(
                        "(p tt) d -> p tt d", p=P
                    ),
                )
                # one streaming pass: cum[p, s, j] = cumsum over the flat
                # 4096-elem stream of kt[p]*mids[b]
                cum = cumpool.tile([P, TT, D], F32, tag="cum")
                nc.vector._custom_dve(
                    PRODUCT_CUMSUM,
                    out=cum[:],
                    in0=kt[:],
                    in1=mb_ps[:, b_local, :].unsqueeze(1).broadcast_to([P, TT, D]),
                )
                # page-end differences -> the 16 dot products of this tile
                c0 = b_local * 32 + h * 16
                ce = cum[:, :, D - 1:D].rearrange("p s o -> p (s o)")  # [P, TT]
                nc.scalar.activation(
                    out=scores[:, c0:c0 + 1], in_=ce[:, 0:1], func=AF.Copy
                )
                nc.vector.tensor_tensor(
                    out=scores[:, c0 + 1:c0 + TT],
                    in0=ce[:, 1:TT],
                    in1=ce[:, 0:TT - 1],
                    op=ALU.subtract,
                )

        # queue the next pair's mb matmuls ahead of this pair's epilogue PE ops
        mb_next = emit_mb(g + 1) if g + 1 < BL // 2 else None

        # ---- epilogue for this pair of batches ----
        th = epool.tile([P, 64], F32, tag="th")
        nc.scalar.activation(out=th[:], in_=scores[:], func=AF.Tanh,
                             bias=bias_col[:], scale=1.0)
        ex = epool.tile([P, 64], F32, tag="ex")
        nc.scalar.activation(out=ex[:], in_=th[:], func=AF.Exp)
        pse = ps_e.tile([64, P], F32)
        nc.tensor.transpose(pse[:], ex[:], ident[:])

        ee = epool.tile([64, P], F32, tag="ee")
        rs = epool.tile([64, 1], F32, tag="rs")
        nc.vector.scalar_tensor_tensor(
            out=ee[:], in0=pse[:], scalar=0.0, in1=mt[:],
            op0=ALU.bypass, op1=ALU.mult, accum_out=rs[:],
        )
        pss = ps_misc.tile([2, 1], F32, tag="mix")
        nc.tensor.matmul(pss[:], lhsT=blk[:], rhs=rs[:], start=True, stop=True)
        rc = epool.tile([2, 1], F32, tag="rc")
        nc.vector.reciprocal(rc[:], pss[:])
        psr2 = ps_misc.tile([64, 1], F32, tag="mix")
        nc.tensor.matmul(psr2[:], lhsT=sel[:], rhs=rc[:], start=True, stop=True)
        rcol = epool.tile([64, 1], F32, tag="rcol")
        nc.vector.tensor_copy(rcol[:], psr2[:])
        attn = epool.tile([64, P], F32, tag="attn")
        nc.scalar.activation(out=attn[:], in_=ee[:], func=AF.Copy, scale=rcol[:])
        nc.gpsimd.dma_start(
            out=out.ap()[g * 2:(g + 1) * 2].rearrange("b c p -> (b c) p"),
            in_=attn[:],
        )
        mb_ps = mb_next


def _install_ntff_hook_shim():
    """Provide antenv.axon_hooks via ctypes into libaxon_pjrt.so (the agent
    image's antenv stub lacks it), enabling NTFF capture under trace=True."""
    import sys
    import types
    import ctypes
    import contextlib

    if "antenv.axon_hooks" in sys.modules:
        return
    so = "/opt/axon/libaxon_pjrt.so"
    if not os.path.exists(so):
        return
    lib = ctypes.CDLL(so)
    if not hasattr(lib, "axon_start_nrt_profile"):
        return
    lib.axon_start_nrt_profile.argtypes = [
        ctypes.POINTER(ctypes.c_int64), ctypes.c_size_t,
    ]
    lib.axon_start_nrt_profile.restype = ctypes.c_int64
    lib.axon_stop_nrt_profile.argtypes = [ctypes.c_char_p]
    lib.axon_stop_nrt_profile.restype = ctypes.c_int64

    @contextlib.contextmanager
    def _hook(output_dir, device_ids):
        import jax

        jax.devices()
        if device_ids:
            ids = (ctypes.c_int64 * len(device_ids))(*device_ids)
            rc = lib.axon_start_nrt_profile(ids, len(device_ids))
        else:
            rc = lib.axon_start_nrt_profile(None, 0)
        if rc != 0:
            raise RuntimeError(f"axon_start_nrt_profile rc={rc}")
        try:
            yield
        finally:
            n = lib.axon_stop_nrt_profile(str(output_dir).encode())
            print(f"profile: {n} file(s) written to {output_dir}", file=sys.stderr)

    mod = types.ModuleType("antenv.axon_hooks")
    mod.get_axon_ntff_profile_hook = lambda: _hook
    mod.set_axon_ntff_profile_hook = lambda h: None
    import antenv

    sys.modules["antenv.axon_hooks"] = mod
    antenv.axon_hooks = mod


_CACHE = {}


def _get_nc():
    if "nc" not in _CACHE:
        from contextlib import ExitStack

        nc = bacc.Bacc("TRN2", debug=False)
        ins = {
            "q": nc.dram_tensor("q", [BL, D], F32, kind="ExternalInput"),
            "k": nc.dram_tensor("k", [BL, T, D], F32, kind="ExternalInput"),
            "mp": nc.dram_tensor("mp", [BL, 32, P], F32, kind="ExternalInput"),
            "W": nc.dram_tensor("W", [D, D], F32, kind="ExternalInput"),
            "bias": nc.dram_tensor("bias", [1, 1], F32, kind="ExternalInput"),
        }
        outs = {"out": nc.dram_tensor("out", [BL, 32, P], F32, kind="ExternalOutput")}
        with tile.TileContext(nc) as tc:
            with ExitStack() as ctx:
                _build_kernel(ctx, tc, outs, ins)
        nc.compile()
        _CACHE["nc"] = nc
    return _CACHE["nc"]


def kernel(q, k, m, W, bias):
    global LAST_RESULTS
    q = np.ascontiguousarray(q, dtype=np.float32)
    k = np.ascontiguousarray(k, dtype=np.float32)
    m = np.ascontiguousarray(m, dtype=np.float32)
    W = np.ascontiguousarray(W, dtype=np.float32)
    bias = np.ascontiguousarray(bias, dtype=np.float32).reshape(1, 1)

    # host-side input marshalling: permute m to the kernel's score layout.
    # mp[b, h*16+s, p] = m[b, h*2048 + p*16 + s]
    mp = np.ascontiguousarray(
        m.reshape(B, H, P, TT).transpose(0, 1, 3, 2).reshape(B, H * TT, P)
    )
    trace = bool(int(os.environ.get("KERNEL_TRACE", "0")))
    if trace:
        _install_ntff_hook_shim()
    nc = _get_nc()
    in_maps = [
        {
            "q": q[i * BL:(i + 1) * BL],
            "k": k[i * BL:(i + 1) * BL],
            "mp": mp[i * BL:(i + 1) * BL],
            "W": W,
            "bias": bias,
        }
        for i in range(NCORES)
    ]
    res = run_bass_kernel_spmd(
        nc,
        in_maps,
        core_ids=list(range(NCORES)),
        trace=trace,
    )
    LAST_RESULTS = res

    full = np.concatenate([res.results[i]["out"] for i in range(NCORES)], axis=0)
    # inverse permutation back to natural [B, T]
    out = np.ascontiguousarray(
        full.reshape(B, H, TT, P).transpose(0, 1, 3, 2).reshape(B, T)
    )
    return out


# revision 30
# speedup vs baseline: 1.1457x; 1.1457x over previous
"""Trainium2 Bass kernel for nn_Attention_17738214932808.

Computation (per batch b):
    mids   = q @ W.T                               [B, D]
    scores = tanh(k . mids + bias)                 [B, T]
    attn   = softmax-with-mask:  e = exp(scores - max) * m ; attn = e / sum(e)
Since tanh is bounded in (-1, 1), the max-subtraction is a mathematical no-op
for the final ratio (exp(s-c)/sum m exp(s-c) is invariant in c), so we compute
e = exp(scores) * m directly; fp32-rounding-level difference only.

Sharding: data-parallel over batch, 8 batches per NeuronCore x 8 cores.

Layout trick: each SBUF partition loads a CONTIGUOUS 16KB run of k (16 t-rows),
so k's DMA runs at ~HBM peak. The resulting score-column permutation is fixed
up by pre-permuting m and inverse-permuting the output on the host (pure input
marshalling; all FLOPs happen on-device).

Dot-product engine: a single custom DVE op (PRODUCT_CUMSUM_ANT) computes the
running prefix sum of k*mids over each partition's whole 4096-element stream
in ONE instruction per [128, 16, 256] tile. The 16 per-partition dot products
are then recovered as differences of the 16 page-end cumsum values (2 tiny DVE
ops per tile). This replaces ~300 per-subtile multiply/reduce instructions
(DVE ~118us + ACT ~116us busy in the previous version) with 16 full-rate
streaming instructions (~70us DVE busy), dropping below the ~94us/core HBM
floor for the 33.5MB k-slice. Numerics: cumsum cancellation error is ~1e-4
absolute on dots of magnitude ~1e2 (fp32 state), far inside tolerance.
"""

import os

import numpy as np

import concourse.bass as bass
import concourse.tile as tile
from concourse import bacc, mybir
from concourse.bass_utils import run_bass_kernel_spmd
from concourse.masks import make_identity

F32 = mybir.dt.float32
AF = mybir.ActivationFunctionType
ALU = mybir.AluOpType

B, T, D = 64, 4096, 256
NCORES = 8
BL = B // NCORES          # batches per core = 8
H = 2                     # halves of T per batch (macro tiles)
TT = 16                   # t-pages per macro  (T = H * 128 * TT)
P = 128

LAST_RESULTS = None       # BassKernelResults of the most recent run (for test.py)


# ---------------------------------------------------------------------------
# Custom DVE op: out[p, :] = cumsum(in0[p, :] * in1[p, :]) along the free dim.
# Registered once at import; sha pins are computed locally so compile()'s
# golden check always matches.
# ---------------------------------------------------------------------------
def _register_product_cumsum():
    import concourse.dve_ops as dve_ops
    from concourse.dve_spec import Spec, Src0, Src1, scan, AluOp, lower
    from concourse.dve_uop import DveOpSpec

    name = "PRODUCT_CUMSUM_ANT"
    if name in dve_ops._SUB_OPCODE_FOR_NAME:
        return next(op for op in dve_ops.OPS if op.name == name)

    def _ref(in0, in1, s0, s1, imm2):
        p = (np.asarray(in0, np.float32) * np.asarray(in1, np.float32))
        flat = p.reshape(p.shape[0], -1)
        return np.cumsum(flat, axis=1, dtype=np.float32).reshape(p.shape)

    spec = Spec(body=scan(AluOp.ADD, Src0 * Src1), reference=_ref)
    shas = {}
    for ver in ("v3", "v4"):
        shas[ver] = DveOpSpec(
            name=name, uops=lower(spec, ver=ver), rd1_en=True
        ).sha(ver)
    op = dve_ops.DveOp(name, spec, subdim=False, uops_sha=shas)
    row = dve_ops._CUSTOM_DVE_ROW_BASE + len(dve_ops.OPS)
    assert row < 0x20, "custom DVE opcode rows exhausted"
    dve_ops.OPS.append(op)
    dve_ops._SUB_OPCODE_FOR_NAME[name] = row
    dve_ops.CUSTOM_DVE_SPECS[name] = spec
    return op


PRODUCT_CUMSUM = _register_product_cumsum()


def _broadcast_row(ap, nparts):
    """[1, N] AP -> [nparts, N] AP with partition step 0."""
    try:
        return ap.to_broadcast([nparts] + list(ap.shape[1:]))
    except Exception:
        return bass.AP(
            tensor=ap.tensor,
            offset=ap.offset,
            ap=[[0, nparts]] + [list(d) for d in ap.ap[1:]],
        )


def _build_kernel(ctx, tc, outs, ins):
    nc = tc.nc
    q, k, mp, W, bias = ins["q"], ins["k"], ins["mp"], ins["W"], ins["bias"]
    out = outs["out"]

    consts = ctx.enter_context(tc.tile_pool(name="consts", bufs=1))
    setup = ctx.enter_context(tc.tile_pool(name="setup", bufs=2))
    kpool = ctx.enter_context(tc.tile_pool(name="kpool", bufs=5))
    cumpool = ctx.enter_context(tc.tile_pool(name="cumpool", bufs=2))
    mtpool = ctx.enter_context(tc.tile_pool(name="mtpool", bufs=4))
    scpool = ctx.enter_context(tc.tile_pool(name="scores", bufs=3))
    epool = ctx.enter_context(tc.tile_pool(name="epil", bufs=2))
    ps_misc = ctx.enter_context(tc.tile_pool(name="ps_misc", bufs=4, space="PSUM"))
    ps_e = ctx.enter_context(tc.tile_pool(name="ps_e", bufs=2, space="PSUM"))

    # ---------------- Phase 0: constants + mids = q @ W.T ----------------
    # W/q go FIRST on the sync (HWDGE) queue: the k stream is one in-order
    # queue striped across the DMA engines, and any other queue gets starved
    # once k is streaming, so in-order-ahead-of-k is the fastest delivery.
    w_sb = setup.tile([P, 2, D], F32, tag="w")
    nc.sync.dma_start(out=w_sb[:], in_=W.ap().rearrange("(dc p) e -> p dc e", p=P))
    q_sb = setup.tile([BL, D], F32, tag="q")
    nc.sync.dma_start(out=q_sb[:], in_=q.ap())

    ident = consts.tile([P, P], F32)
    make_identity(nc, ident)

    bias_col = consts.tile([P, 1], F32)
    nc.gpsimd.dma_start(out=bias_col[:], in_=_broadcast_row(bias.ap(), P))

    # ones block-diagonal [64, 2]: blk[p, g] = 1 iff p//32 == g
    blk = consts.tile([64, 2], F32)
    nc.gpsimd.memset(blk[:], 1.0)
    nc.gpsimd.affine_select(   # keep where p - 32g >= 0
        out=blk[:], in_=blk[:], compare_op=ALU.is_ge, fill=0.0,
        base=0, pattern=[[-32, 2]], channel_multiplier=1,
    )
    nc.gpsimd.affine_select(   # keep where 31 - p + 32g >= 0  (i.e. p - 32g <= 31)
        out=blk[:], in_=blk[:], compare_op=ALU.is_ge, fill=0.0,
        base=31, pattern=[[32, 2]], channel_multiplier=-1,
    )
    # selector [2, 64]: sel[g, x] = 1 iff x//32 == g
    sel = consts.tile([2, 64], F32)
    nc.gpsimd.memset(sel[:], 0.0)
    nc.gpsimd.affine_select(   # iota = g - (x//32); equal -> fill... keep where != 0
        out=sel.rearrange("p (g x) -> p g x", g=2),
        in_=sel.rearrange("p (g x) -> p g x", g=2),
        compare_op=ALU.not_equal, fill=1.0,
        base=0, pattern=[[-1, 2], [0, 32]], channel_multiplier=1,
    )
    # W^T chunks via PE transposes, all into ONE psum bank -> one DVE copy.
    # wt[p=e_local, i=(dc, ec), d_local]
    pst_w = ps_misc.tile([P, 4, P], F32, tag="mix")
    for dc in range(2):
        for ec in range(2):
            nc.tensor.transpose(
                pst_w[:, dc * 2 + ec, :], w_sb[:, dc, ec * P:(ec + 1) * P], ident[:]
            )
    wt = setup.tile([P, 4, P], F32, tag="wt")
    nc.vector.tensor_copy(wt[:], pst_w[:])
    # q^T chunks: qt[p=e_local, ec, b]
    pst_q = ps_misc.tile([P, 2, BL], F32, tag="mix")
    for ec in range(2):
        nc.tensor.transpose(
            pst_q[:, ec, :], q_sb[:, ec * P:(ec + 1) * P], ident[0:BL, 0:BL]
        )
    qt = setup.tile([P, 2, BL], F32, tag="qt")
    nc.vector.tensor_copy(qt[:], pst_q[:])

    # mb[p', d] = mids[b, d] = sum_e W[d, e] q[b, e], computed directly on
    # the PE for all 128 partitions at once: lhsT = qt column b broadcast
    # across the out-partition dim, rhs = the matching W^T chunk. The scans
    # read it straight from PSUM. Pair g+1's pair of mb tiles is emitted
    # right after pair g's scans (before pair g's epilogue PE work) so the
    # next pair's first scan never stalls behind the epilogue.
    ps_mb = ctx.enter_context(tc.tile_pool(name="ps_mb", bufs=2, space="PSUM"))

    def emit_mb(g):
        mb = ps_mb.tile([P, 2, D], F32)
        for b_local in range(2):
            b = g * 2 + b_local
            for dc in range(2):
                for ec in range(2):
                    nc.tensor.matmul(
                        mb[:, b_local, dc * P:(dc + 1) * P],
                        lhsT=qt[:, ec, b:b + 1].broadcast_to([P, P]),
                        rhs=wt[:, dc * 2 + ec, :],
                        start=(ec == 0), stop=(ec == 1),
                    )
        return mb

    mb_ps = emit_mb(0)

    # ---------------- Phase 1: main loop + epilogue per batch-pair ----------------
    for g in range(BL // 2):                 # 4 pairs
        scores = scpool.tile([P, 64], F32)   # col = b_local*32 + h*16 + s
        # prefetch this pair's mask tile (needed only in the epilogue)
        mt = mtpool.tile([64, P], F32, tag="mt")
        nc.gpsimd.dma_start(
            out=mt[:],
            in_=mp.ap()[g * 2:(g + 1) * 2].rearrange("b c p -> (b c) p"),
        )

        for b_local in range(2):
            b = g * 2 + b_local
            for h in range(2):
                kt = kpool.tile([P, TT, D], F32, tag="ktile")
                nc.sync.dma_start(
                    out=kt[:],
                    in_=k.ap()[b, h * 2048:(h + 1) * 2048, :].rearrange(
                        "(p tt) d -> p tt d", p=P
                    ),
                )
                # one streaming pass: cum[p, s, j] = cumsum over the flat
                # 4096-elem stream of kt[p]*mids[b]
                cum = cumpool.tile([P, TT, D], F32, tag="cum")
                nc.vector._custom_dve(
                    PRODUCT_CUMSUM,
                    out=cum[:],
                    in0=kt[:],
                    in1=mb_ps[:, b_local, :].unsqueeze(1).broadcast_to([P, TT, D]),
                )
                # page-end differences -> the 16 dot products of this tile
                c0 = b_local * 32 + h * 16
                ce = cum[:, :, D - 1:D].rearrange("p s o -> p (s o)")  # [P, TT]
                nc.scalar.activation(
                    out=scores[:, c0:c0 + 1], in_=ce[:, 0:1], func=AF.Copy
                )
                nc.vector.tensor_tensor(
                    out=scores[:, c0 + 1:c0 + TT],
                    in0=ce[:, 1:TT],
                    in1=ce[:, 0:TT - 1],
                    op=ALU.subtract,
                )

        # queue the next pair's mb matmuls ahead of this pair's epilogue PE ops
        mb_next = emit_mb(g + 1) if g + 1 < BL // 2 else None

        # ---- epilogue for this pair of batches ----
        th = epool.tile([P, 64], F32, tag="th")
        nc.scalar.activation(out=th[:], in_=scores[:], func=AF.Tanh,
                             bias=bias_col[:], scale=1.0)
        ex = epool.tile([P, 64], F32, tag="ex")
        nc.scalar.activation(out=ex[:], in_=th[:], func=AF.Exp)
        pse = ps_e.tile([64, P], F32)
        nc.tensor.transpose(pse[:], ex[:], ident[:])

        ee = epool.tile([64, P], F32, tag="ee")
        rs = epool.tile([64, 1], F32, tag="rs")
        nc.vector.scalar_tensor_tensor(
            out=ee[:], in0=pse[:], scalar=0.0, in1=mt[:],
            op0=ALU.bypass, op1=ALU.mult, accum_out=rs[:],
        )
        pss = ps_misc.tile([2, 1], F32, tag="mix")
        nc.tensor.matmul(pss[:], lhsT=blk[:], rhs=rs[:], start=True, stop=True)
        rc = epool.tile([2, 1], F32, tag="rc")
        nc.vector.reciprocal(rc[:], pss[:])
        psr2 = ps_misc.tile([64, 1], F32, tag="mix")
        nc.tensor.matmul(psr2[:], lhsT=sel[:], rhs=rc[:], start=True, stop=True)
        rcol = epool.tile([64, 1], F32, tag="rcol")
        nc.vector.tensor_copy(rcol[:], psr2[:])
        attn = epool.tile([64, P], F32, tag="attn")
        nc.scalar.activation(out=attn[:], in_=ee[:], func=AF.Copy, scale=rcol[:])
        nc.gpsimd.dma_start(
            out=out.ap()[g * 2:(g + 1) * 2].rearrange("b c p -> (b c) p"),
            in_=attn[:],
        )
        mb_ps = mb_next


def _install_ntff_hook_shim():
    """Provide antenv.axon_hooks via ctypes into libaxon_pjrt.so (the agent
    image's antenv stub lacks it), enabling NTFF capture under trace=True."""
    import sys
    import types
    import ctypes
    import contextlib

    if "antenv.axon_hooks" in sys.modules:
        return
    so = "/opt/axon/libaxon_pjrt.so"
    if not os.path.exists(so):
        return
    lib = ctypes.CDLL(so)
    if not hasattr(lib, "axon_start_nrt_profile"):
        return
    lib.axon_start_nrt_profile.argtypes = [
        ctypes.POINTER(ctypes.c_int64), ctypes.c_size_t,
    ]
    lib.axon_start_nrt_profile.restype = ctypes.c_int64
    lib.axon_stop_nrt_profile.argtypes = [ctypes.c_char_p]
    lib.axon_stop_nrt_profile.restype = ctypes.c_int64

    @contextlib.contextmanager
    def _hook(output_dir, device_ids):
        import jax

        jax.devices()
        if device_ids:
            ids = (ctypes.c_int64 * len(device_ids))(*device_ids)
            rc = lib.axon_start_nrt_profile(ids, len(device_ids))
        else:
            rc = lib.axon_start_nrt_profile(None, 0)
        if rc != 0:
            raise RuntimeError(f"axon_start_nrt_profile rc={rc}")
        try:
            yield
        finally:
            n = lib.axon_stop_nrt_profile(str(output_dir).encode())
            print(f"profile: {n} file(s) written to {output_dir}", file=sys.stderr)

    mod = types.ModuleType("antenv.axon_hooks")
    mod.get_axon_ntff_profile_hook = lambda: _hook
    mod.set_axon_ntff_profile_hook = lambda h: None
    import antenv

    sys.modules["antenv.axon_hooks"] = mod
    antenv.axon_hooks = mod


_CACHE = {}


def _get_nc():
    if "nc" not in _CACHE:
        from contextlib import ExitStack

        nc = bacc.Bacc("TRN2", debug=False)
        ins = {
            "q": nc.dram_tensor("q", [BL, D], F32, kind="ExternalInput"),
            "k": nc.dram_tensor("k", [BL, T, D], F32, kind="ExternalInput"),
            "mp": nc.dram_tensor("mp", [BL, 32, P], F32, kind="ExternalInput"),
            "W": nc.dram_tensor("W", [D, D], F32, kind="ExternalInput"),
            "bias": nc.dram_tensor("bias", [1, 1], F32, kind="ExternalInput"),
        }
        outs = {"out": nc.dram_tensor("out", [BL, 32, P], F32, kind="ExternalOutput")}
        with tile.TileContext(nc) as tc:
            with ExitStack() as ctx:
                _build_kernel(ctx, tc, outs, ins)
        nc.compile()
        _CACHE["nc"] = nc
    return _CACHE["nc"]


def kernel(q, k, m, W, bias):
    global LAST_RESULTS
    q = np.ascontiguousarray(q, dtype=np.float32)
    k = np.ascontiguousarray(k, dtype=np.float32)
    m = np.ascontiguousarray(m, dtype=np.float32)
    W = np.ascontiguousarray(W, dtype=np.float32)
    bias = np.ascontiguousarray(bias, dtype=np.float32).reshape(1, 1)

    # host-side input marshalling: permute m to the kernel's score layout.
    # mp[b, h*16+s, p] = m[b, h*2048 + p*16 + s]
    mp = np.ascontiguousarray(
        m.reshape(B, H, P, TT).transpose(0, 1, 3, 2).reshape(B, H * TT, P)
    )
    trace = bool(int(os.environ.get("KERNEL_TRACE", "0")))
    if trace:
        _install_ntff_hook_shim()
    nc = _get_nc()
    in_maps = [
        {
            "q": q[i * BL:(i + 1) * BL],
            "k": k[i * BL:(i + 1) * BL],
            "mp": mp[i * BL:(i + 1) * BL],
            "W": W,
            "bias": bias,
        }
        for i in range(NCORES)
    ]
    res = run_bass_kernel_spmd(
        nc,
        in_maps,
        core_ids=list(range(NCORES)),
        trace=trace,
    )
    LAST_RESULTS = res

    full = np.concatenate([res.results[i]["out"] for i in range(NCORES)], axis=0)
    # inverse permutation back to natural [B, T]
    out = np.ascontiguousarray(
        full.reshape(B, H, TT, P).transpose(0, 1, 3, 2).reshape(B, T)
    )
    return out


# revision 32
# speedup vs baseline: 1.1563x; 1.0093x over previous
"""Trainium2 Bass kernel for nn_Attention_17738214932808.

Computation (per batch b):
    mids   = q @ W.T                               [B, D]
    scores = tanh(k . mids + bias)                 [B, T]
    attn   = softmax-with-mask:  e = exp(scores - max) * m ; attn = e / sum(e)
Since tanh is bounded in (-1, 1), the max-subtraction is a mathematical no-op
for the final ratio (exp(s-c)/sum m exp(s-c) is invariant in c), so we compute
e = exp(scores) * m directly; fp32-rounding-level difference only.

Sharding: data-parallel over batch, 8 batches per NeuronCore x 8 cores.

Layout trick: each SBUF partition loads a CONTIGUOUS 16KB run of k (16 t-rows),
so k's DMA runs at ~HBM peak. The resulting score-column permutation is fixed
up by pre-permuting m and inverse-permuting the output on the host (pure input
marshalling; all FLOPs happen on-device).

Dot-product engine: a single custom DVE op (PRODUCT_CUMSUM_ANT) computes the
running prefix sum of k*mids over each partition's whole 4096-element stream
in ONE instruction per [128, 16, 256] tile. The 16 per-partition dot products
are then recovered as differences of the 16 page-end cumsum values (2 tiny DVE
ops per tile). This replaces ~300 per-subtile multiply/reduce instructions
(DVE ~118us + ACT ~116us busy in the previous version) with 16 full-rate
streaming instructions (~70us DVE busy), dropping below the ~94us/core HBM
floor for the 33.5MB k-slice. Numerics: cumsum cancellation error is ~1e-4
absolute on dots of magnitude ~1e2 (fp32 state), far inside tolerance.
"""

import os

import numpy as np

import concourse.bass as bass
import concourse.tile as tile
from concourse import bacc, mybir
from concourse.bass_utils import run_bass_kernel_spmd
from concourse.masks import make_identity

F32 = mybir.dt.float32
AF = mybir.ActivationFunctionType
ALU = mybir.AluOpType

B, T, D = 64, 4096, 256
NCORES = 8
BL = B // NCORES          # batches per core = 8
H = 2                     # halves of T per batch (macro tiles)
TT = 16                   # t-pages per macro  (T = H * 128 * TT)
P = 128

LAST_RESULTS = None       # BassKernelResults of the most recent run (for test.py)


# ---------------------------------------------------------------------------
# Custom DVE op: out[p, :] = cumsum(in0[p, :] * in1[p, :]) along the free dim.
# Registered once at import; sha pins are computed locally so compile()'s
# golden check always matches.
# ---------------------------------------------------------------------------
def _register_product_cumsum():
    import concourse.dve_ops as dve_ops
    from concourse.dve_spec import Spec, Src0, Src1, scan, AluOp, lower
    from concourse.dve_uop import DveOpSpec

    name = "PRODUCT_CUMSUM_ANT"
    if name in dve_ops._SUB_OPCODE_FOR_NAME:
        return next(op for op in dve_ops.OPS if op.name == name)

    def _ref(in0, in1, s0, s1, imm2):
        p = (np.asarray(in0, np.float32) * np.asarray(in1, np.float32))
        flat = p.reshape(p.shape[0], -1)
        return np.cumsum(flat, axis=1, dtype=np.float32).reshape(p.shape)

    spec = Spec(body=scan(AluOp.ADD, Src0 * Src1), reference=_ref)
    shas = {}
    for ver in ("v3", "v4"):
        shas[ver] = DveOpSpec(
            name=name, uops=lower(spec, ver=ver), rd1_en=True
        ).sha(ver)
    op = dve_ops.DveOp(name, spec, subdim=False, uops_sha=shas)
    row = dve_ops._CUSTOM_DVE_ROW_BASE + len(dve_ops.OPS)
    assert row < 0x20, "custom DVE opcode rows exhausted"
    dve_ops.OPS.append(op)
    dve_ops._SUB_OPCODE_FOR_NAME[name] = row
    dve_ops.CUSTOM_DVE_SPECS[name] = spec
    return op


PRODUCT_CUMSUM = _register_product_cumsum()


def _broadcast_row(ap, nparts):
    """[1, N] AP -> [nparts, N] AP with partition step 0."""
    try:
        return ap.to_broadcast([nparts] + list(ap.shape[1:]))
    except Exception:
        return bass.AP(
            tensor=ap.tensor,
            offset=ap.offset,
            ap=[[0, nparts]] + [list(d) for d in ap.ap[1:]],
        )


def _build_kernel(ctx, tc, outs, ins):
    nc = tc.nc
    q, k, mp, W, bias = ins["q"], ins["k"], ins["mp"], ins["W"], ins["bias"]
    out = outs["out"]

    consts = ctx.enter_context(tc.tile_pool(name="consts", bufs=1))
    setup = ctx.enter_context(tc.tile_pool(name="setup", bufs=2))
    kpool = ctx.enter_context(tc.tile_pool(name="kpool", bufs=4))
    cumpool = ctx.enter_context(tc.tile_pool(name="cumpool", bufs=2))
    kpool_s = ctx.enter_context(tc.tile_pool(name="kpool_s", bufs=2))
    cumpool_s = ctx.enter_context(tc.tile_pool(name="cumpool_s", bufs=2))
    mtpool = ctx.enter_context(tc.tile_pool(name="mtpool", bufs=4))
    scpool = ctx.enter_context(tc.tile_pool(name="scores", bufs=3))
    epool = ctx.enter_context(tc.tile_pool(name="epil", bufs=2))
    ps_misc = ctx.enter_context(tc.tile_pool(name="ps_misc", bufs=4, space="PSUM"))
    ps_e = ctx.enter_context(tc.tile_pool(name="ps_e", bufs=2, space="PSUM"))

    # ---------------- Phase 0: constants + mids = q @ W.T ----------------
    # W/q go FIRST on the sync (HWDGE) queue: the k stream is one in-order
    # queue striped across the DMA engines, and any other queue gets starved
    # once k is streaming, so in-order-ahead-of-k is the fastest delivery.
    w_sb = setup.tile([P, 2, D], F32, tag="w")
    nc.sync.dma_start(out=w_sb[:], in_=W.ap().rearrange("(dc p) e -> p dc e", p=P))
    q_sb = setup.tile([BL, D], F32, tag="q")
    nc.sync.dma_start(out=q_sb[:], in_=q.ap())

    ident = consts.tile([P, P], F32)
    make_identity(nc, ident)

    bias_col = consts.tile([P, 1], F32)
    nc.gpsimd.dma_start(out=bias_col[:], in_=_broadcast_row(bias.ap(), P))

    # ones block-diagonal [64, 2]: blk[p, g] = 1 iff p//32 == g
    blk = consts.tile([64, 2], F32)
    nc.gpsimd.memset(blk[:], 1.0)
    nc.gpsimd.affine_select(   # keep where p - 32g >= 0
        out=blk[:], in_=blk[:], compare_op=ALU.is_ge, fill=0.0,
        base=0, pattern=[[-32, 2]], channel_multiplier=1,
    )
    nc.gpsimd.affine_select(   # keep where 31 - p + 32g >= 0  (i.e. p - 32g <= 31)
        out=blk[:], in_=blk[:], compare_op=ALU.is_ge, fill=0.0,
        base=31, pattern=[[32, 2]], channel_multiplier=-1,
    )
    # selector [2, 64]: sel[g, x] = 1 iff x//32 == g
    sel = consts.tile([2, 64], F32)
    nc.gpsimd.memset(sel[:], 0.0)
    nc.gpsimd.affine_select(   # iota = g - (x//32); equal -> fill... keep where != 0
        out=sel.rearrange("p (g x) -> p g x", g=2),
        in_=sel.rearrange("p (g x) -> p g x", g=2),
        compare_op=ALU.not_equal, fill=1.0,
        base=0, pattern=[[-1, 2], [0, 32]], channel_multiplier=1,
    )
    # W^T chunks via PE transposes, all into ONE psum bank -> one DVE copy.
    # wt[p=e_local, i=(dc, ec), d_local]
    pst_w = ps_misc.tile([P, 4, P], F32, tag="mix")
    for dc in range(2):
        for ec in range(2):
            nc.tensor.transpose(
                pst_w[:, dc * 2 + ec, :], w_sb[:, dc, ec * P:(ec + 1) * P], ident[:]
            )
    wt = setup.tile([P, 4, P], F32, tag="wt")
    nc.vector.tensor_copy(wt[:], pst_w[:])
    # q^T chunks: qt[p=e_local, ec, b]
    pst_q = ps_misc.tile([P, 2, BL], F32, tag="mix")
    for ec in range(2):
        nc.tensor.transpose(
            pst_q[:, ec, :], q_sb[:, ec * P:(ec + 1) * P], ident[0:BL, 0:BL]
        )
    qt = setup.tile([P, 2, BL], F32, tag="qt")
    nc.vector.tensor_copy(qt[:], pst_q[:])

    # mb[p', d] = mids[b, d] = sum_e W[d, e] q[b, e], computed directly on
    # the PE for all 128 partitions at once: lhsT = qt column b broadcast
    # across the out-partition dim, rhs = the matching W^T chunk. The scans
    # read it straight from PSUM. Pair g+1's pair of mb tiles is emitted
    # right after pair g's scans (before pair g's epilogue PE work) so the
    # next pair's first scan never stalls behind the epilogue.
    ps_mb = ctx.enter_context(tc.tile_pool(name="ps_mb", bufs=2, space="PSUM"))

    def emit_mb(g):
        mb = ps_mb.tile([P, 2, D], F32)
        for b_local in range(2):
            b = g * 2 + b_local
            for dc in range(2):
                for ec in range(2):
                    nc.tensor.matmul(
                        mb[:, b_local, dc * P:(dc + 1) * P],
                        lhsT=qt[:, ec, b:b + 1].broadcast_to([P, P]),
                        rhs=wt[:, dc * 2 + ec, :],
                        start=(ec == 0), stop=(ec == 1),
                    )
        return mb

    mb_ps = emit_mb(0)

    # ---------------- Phase 1: main loop + epilogue per batch-pair ----------------
    for g in range(BL // 2):                 # 4 pairs
        scores = scpool.tile([P, 64], F32)   # col = b_local*32 + h*16 + s
        # prefetch this pair's mask tile (needed only in the epilogue)
        mt = mtpool.tile([64, P], F32, tag="mt")
        nc.gpsimd.dma_start(
            out=mt[:],
            in_=mp.ap()[g * 2:(g + 1) * 2].rearrange("b c p -> (b c) p"),
        )

        for b_local in range(2):
            b = g * 2 + b_local
            for h in range(2):
                c0 = b_local * 32 + h * 16
                # the very first and very last macros are split into two
                # half-tiles: the first so scan 0 starts ~4us earlier (smaller
                # first transfer), the last so the final scan after the stream
                # ends is 2.2us instead of 4.4us
                split = (g == 0 and b_local == 0 and h == 0) or (
                    g == 3 and b_local == 1 and h == 1
                )
                if split:
                    src = k.ap()[b, h * 2048:(h + 1) * 2048, :].rearrange(
                        "(p half tt) d -> half p tt d", p=P, half=2
                    )
                    subs = [(kpool_s, cumpool_s, TT // 2, src[0]),
                            (kpool_s, cumpool_s, TT // 2, src[1])]
                else:
                    subs = [(kpool, cumpool, TT,
                             k.ap()[b, h * 2048:(h + 1) * 2048, :].rearrange(
                                 "(p tt) d -> p tt d", p=P))]
                for si, (kp, cp, tts, src_ap) in enumerate(subs):
                    kt = kp.tile([P, tts, D], F32, tag="ktile")
                    nc.sync.dma_start(out=kt[:], in_=src_ap)
                    # one streaming pass: cum[p, s, j] = cumsum over the flat
                    # per-partition stream of kt[p]*mids[b]
                    cum = cp.tile([P, tts, D], F32, tag="cum")
                    nc.vector._custom_dve(
                        PRODUCT_CUMSUM,
                        out=cum[:],
                        in0=kt[:],
                        in1=mb_ps[:, b_local, :].unsqueeze(1).broadcast_to(
                            [P, tts, D]
                        ),
                    )
                    # page-end differences -> the dot products of this tile
                    cc = c0 + si * tts
                    ce = cum[:, :, D - 1:D].rearrange("p s o -> p (s o)")
                    nc.vector.tensor_copy(scores[:, cc:cc + 1], ce[:, 0:1])
                    nc.vector.tensor_tensor(
                        out=scores[:, cc + 1:cc + tts],
                        in0=ce[:, 1:tts],
                        in1=ce[:, 0:tts - 1],
                        op=ALU.subtract,
                    )

        # queue the next pair's mb matmuls ahead of this pair's epilogue PE ops
        mb_next = emit_mb(g + 1) if g + 1 < BL // 2 else None

        # ---- epilogue for this pair of batches ----
        th = epool.tile([P, 64], F32, tag="th")
        nc.scalar.activation(out=th[:], in_=scores[:], func=AF.Tanh,
                             bias=bias_col[:], scale=1.0)
        ex = epool.tile([P, 64], F32, tag="ex")
        nc.scalar.activation(out=ex[:], in_=th[:], func=AF.Exp)
        pse = ps_e.tile([64, P], F32)
        nc.tensor.transpose(pse[:], ex[:], ident[:])

        ee = epool.tile([64, P], F32, tag="ee")
        rs = epool.tile([64, 1], F32, tag="rs")
        nc.vector.scalar_tensor_tensor(
            out=ee[:], in0=pse[:], scalar=0.0, in1=mt[:],
            op0=ALU.bypass, op1=ALU.mult, accum_out=rs[:],
        )
        pss = ps_misc.tile([2, 1], F32, tag="mix")
        nc.tensor.matmul(pss[:], lhsT=blk[:], rhs=rs[:], start=True, stop=True)
        rc = epool.tile([2, 1], F32, tag="rc")
        nc.vector.reciprocal(rc[:], pss[:])
        psr2 = ps_misc.tile([64, 1], F32, tag="mix")
        nc.tensor.matmul(psr2[:], lhsT=sel[:], rhs=rc[:], start=True, stop=True)
        rcol = epool.tile([64, 1], F32, tag="rcol")
        nc.vector.tensor_copy(rcol[:], psr2[:])
        attn = epool.tile([64, P], F32, tag="attn")
        nc.scalar.activation(out=attn[:], in_=ee[:], func=AF.Copy, scale=rcol[:])
        nc.gpsimd.dma_start(
            out=out.ap()[g * 2:(g + 1) * 2].rearrange("b c p -> (b c) p"),
            in_=attn[:],
        )
        mb_ps = mb_next


def _install_ntff_hook_shim():
    """Provide antenv.axon_hooks via ctypes into libaxon_pjrt.so (the agent
    image's antenv stub lacks it), enabling NTFF capture under trace=True."""
    import sys
    import types
    import ctypes
    import contextlib

    if "antenv.axon_hooks" in sys.modules:
        return
    so = "/opt/axon/libaxon_pjrt.so"
    if not os.path.exists(so):
        return
    lib = ctypes.CDLL(so)
    if not hasattr(lib, "axon_start_nrt_profile"):
        return
    lib.axon_start_nrt_profile.argtypes = [
        ctypes.POINTER(ctypes.c_int64), ctypes.c_size_t,
    ]
    lib.axon_start_nrt_profile.restype = ctypes.c_int64
    lib.axon_stop_nrt_profile.argtypes = [ctypes.c_char_p]
    lib.axon_stop_nrt_profile.restype = ctypes.c_int64

    @contextlib.contextmanager
    def _hook(output_dir, device_ids):
        import jax

        jax.devices()
        if device_ids:
            ids = (ctypes.c_int64 * len(device_ids))(*device_ids)
            rc = lib.axon_start_nrt_profile(ids, len(device_ids))
        else:
            rc = lib.axon_start_nrt_profile(None, 0)
        if rc != 0:
            raise RuntimeError(f"axon_start_nrt_profile rc={rc}")
        try:
            yield
        finally:
            n = lib.axon_stop_nrt_profile(str(output_dir).encode())
            print(f"profile: {n} file(s) written to {output_dir}", file=sys.stderr)

    mod = types.ModuleType("antenv.axon_hooks")
    mod.get_axon_ntff_profile_hook = lambda: _hook
    mod.set_axon_ntff_profile_hook = lambda h: None
    import antenv

    sys.modules["antenv.axon_hooks"] = mod
    antenv.axon_hooks = mod


_CACHE = {}


def _get_nc():
    if "nc" not in _CACHE:
        from contextlib import ExitStack

        nc = bacc.Bacc("TRN2", debug=False)
        ins = {
            "q": nc.dram_tensor("q", [BL, D], F32, kind="ExternalInput"),
            "k": nc.dram_tensor("k", [BL, T, D], F32, kind="ExternalInput"),
            "mp": nc.dram_tensor("mp", [BL, 32, P], F32, kind="ExternalInput"),
            "W": nc.dram_tensor("W", [D, D], F32, kind="ExternalInput"),
            "bias": nc.dram_tensor("bias", [1, 1], F32, kind="ExternalInput"),
        }
        outs = {"out": nc.dram_tensor("out", [BL, 32, P], F32, kind="ExternalOutput")}
        with tile.TileContext(nc) as tc:
            with ExitStack() as ctx:
                _build_kernel(ctx, tc, outs, ins)
        nc.compile()
        _CACHE["nc"] = nc
    return _CACHE["nc"]


def kernel(q, k, m, W, bias):
    global LAST_RESULTS
    q = np.ascontiguousarray(q, dtype=np.float32)
    k = np.ascontiguousarray(k, dtype=np.float32)
    m = np.ascontiguousarray(m, dtype=np.float32)
    W = np.ascontiguousarray(W, dtype=np.float32)
    bias = np.ascontiguousarray(bias, dtype=np.float32).reshape(1, 1)

    # host-side input marshalling: permute m to the kernel's score layout.
    # mp[b, h*16+s, p] = m[b, h*2048 + p*16 + s]
    mp = np.ascontiguousarray(
        m.reshape(B, H, P, TT).transpose(0, 1, 3, 2).reshape(B, H * TT, P)
    )
    trace = bool(int(os.environ.get("KERNEL_TRACE", "0")))
    if trace:
        _install_ntff_hook_shim()
    nc = _get_nc()
    in_maps = [
        {
            "q": q[i * BL:(i + 1) * BL],
            "k": k[i * BL:(i + 1) * BL],
            "mp": mp[i * BL:(i + 1) * BL],
            "W": W,
            "bias": bias,
        }
        for i in range(NCORES)
    ]
    res = run_bass_kernel_spmd(
        nc,
        in_maps,
        core_ids=list(range(NCORES)),
        trace=trace,
    )
    LAST_RESULTS = res

    full = np.concatenate([res.results[i]["out"] for i in range(NCORES)], axis=0)
    # inverse permutation back to natural [B, T]
    out = np.ascontiguousarray(
        full.reshape(B, H, TT, P).transpose(0, 1, 3, 2).reshape(B, T)
    )
    return out


# revision 36
# speedup vs baseline: 1.1564x; 1.0001x over previous
"""Trainium2 Bass kernel for nn_Attention_17738214932808.

Computation (per batch b):
    mids   = q @ W.T                               [B, D]
    scores = tanh(k . mids + bias)                 [B, T]
    attn   = softmax-with-mask:  e = exp(scores - max) * m ; attn = e / sum(e)
Since tanh is bounded in (-1, 1), the max-subtraction is a mathematical no-op
for the final ratio (exp(s-c)/sum m exp(s-c) is invariant in c), so we compute
e = exp(scores) * m directly; fp32-rounding-level difference only.

Sharding: data-parallel over batch, 8 batches per NeuronCore x 8 cores.

Layout trick: each SBUF partition loads a CONTIGUOUS 16KB run of k (16 t-rows),
so k's DMA runs at ~HBM peak. The resulting score-column permutation is fixed
up by pre-permuting m and inverse-permuting the output on the host (pure input
marshalling; all FLOPs happen on-device).

Dot-product engine: a single custom DVE op (PRODUCT_CUMSUM_ANT) computes the
running prefix sum of k*mids over each partition's whole 4096-element stream
in ONE instruction per [128, 16, 256] tile. The 16 per-partition dot products
are then recovered as differences of the 16 page-end cumsum values (2 tiny DVE
ops per tile). This replaces ~300 per-subtile multiply/reduce instructions
(DVE ~118us + ACT ~116us busy in the previous version) with 16 full-rate
streaming instructions (~70us DVE busy), dropping below the ~94us/core HBM
floor for the 33.5MB k-slice. Numerics: cumsum cancellation error is ~1e-4
absolute on dots of magnitude ~1e2 (fp32 state), far inside tolerance.
"""

import os

import numpy as np

import concourse.bass as bass
import concourse.tile as tile
from concourse import bacc, mybir
from concourse.bass_utils import run_bass_kernel_spmd
from concourse.masks import make_identity

F32 = mybir.dt.float32
AF = mybir.ActivationFunctionType
ALU = mybir.AluOpType

B, T, D = 64, 4096, 256
NCORES = 8
BL = B // NCORES          # batches per core = 8
H = 2                     # halves of T per batch (macro tiles)
TT = 16                   # t-pages per macro  (T = H * 128 * TT)
P = 128

LAST_RESULTS = None       # BassKernelResults of the most recent run (for test.py)


# ---------------------------------------------------------------------------
# Custom DVE op: out[p, :] = cumsum(in0[p, :] * in1[p, :]) along the free dim.
# Registered once at import; sha pins are computed locally so compile()'s
# golden check always matches.
# ---------------------------------------------------------------------------
def _register_product_cumsum():
    import concourse.dve_ops as dve_ops
    from concourse.dve_spec import Spec, Src0, Src1, scan, AluOp, lower
    from concourse.dve_uop import DveOpSpec

    name = "PRODUCT_CUMSUM_ANT"
    if name in dve_ops._SUB_OPCODE_FOR_NAME:
        return next(op for op in dve_ops.OPS if op.name == name)

    def _ref(in0, in1, s0, s1, imm2):
        p = (np.asarray(in0, np.float32) * np.asarray(in1, np.float32))
        flat = p.reshape(p.shape[0], -1)
        return np.cumsum(flat, axis=1, dtype=np.float32).reshape(p.shape)

    spec = Spec(body=scan(AluOp.ADD, Src0 * Src1), reference=_ref)
    shas = {}
    for ver in ("v3", "v4"):
        shas[ver] = DveOpSpec(
            name=name, uops=lower(spec, ver=ver), rd1_en=True
        ).sha(ver)
    op = dve_ops.DveOp(name, spec, subdim=False, uops_sha=shas)
    row = dve_ops._CUSTOM_DVE_ROW_BASE + len(dve_ops.OPS)
    assert row < 0x20, "custom DVE opcode rows exhausted"
    dve_ops.OPS.append(op)
    dve_ops._SUB_OPCODE_FOR_NAME[name] = row
    dve_ops.CUSTOM_DVE_SPECS[name] = spec
    return op


PRODUCT_CUMSUM = _register_product_cumsum()


def _broadcast_row(ap, nparts):
    """[1, N] AP -> [nparts, N] AP with partition step 0."""
    try:
        return ap.to_broadcast([nparts] + list(ap.shape[1:]))
    except Exception:
        return bass.AP(
            tensor=ap.tensor,
            offset=ap.offset,
            ap=[[0, nparts]] + [list(d) for d in ap.ap[1:]],
        )


def _build_kernel(ctx, tc, outs, ins):
    nc = tc.nc
    q, k, mp, W, bias = ins["q"], ins["k"], ins["mp"], ins["W"], ins["bias"]
    out = outs["out"]

    consts = ctx.enter_context(tc.tile_pool(name="consts", bufs=1))
    setup = ctx.enter_context(tc.tile_pool(name="setup", bufs=2))
    kpool = ctx.enter_context(tc.tile_pool(name="kpool", bufs=4))
    cumpool = ctx.enter_context(tc.tile_pool(name="cumpool", bufs=2))
    kpool_s = ctx.enter_context(tc.tile_pool(name="kpool_s", bufs=2))
    cumpool_s = ctx.enter_context(tc.tile_pool(name="cumpool_s", bufs=2))
    mtpool = ctx.enter_context(tc.tile_pool(name="mtpool", bufs=4))
    scpool = ctx.enter_context(tc.tile_pool(name="scores", bufs=3))
    epool = ctx.enter_context(tc.tile_pool(name="epil", bufs=2))
    ps_misc = ctx.enter_context(tc.tile_pool(name="ps_misc", bufs=4, space="PSUM"))
    ps_e = ctx.enter_context(tc.tile_pool(name="ps_e", bufs=2, space="PSUM"))

    # ---------------- Phase 0: constants + mids = q @ W.T ----------------
    # W/q go FIRST on the sync (HWDGE) queue: the k stream is one in-order
    # queue striped across the DMA engines, and any other queue gets starved
    # once k is streaming, so in-order-ahead-of-k is the fastest delivery.
    w_sb = setup.tile([P, 2, D], F32, tag="w")
    nc.sync.dma_start(out=w_sb[:], in_=W.ap().rearrange("(dc p) e -> p dc e", p=P))
    q_sb = setup.tile([BL, D], F32, tag="q")
    nc.sync.dma_start(out=q_sb[:], in_=q.ap())

    ident = consts.tile([P, P], F32)
    make_identity(nc, ident)

    bias_col = consts.tile([P, 1], F32)
    nc.gpsimd.dma_start(out=bias_col[:], in_=_broadcast_row(bias.ap(), P))

    # ones block-diagonal [64, 2]: blk[p, g] = 1 iff p//32 == g
    blk = consts.tile([64, 2], F32)
    nc.gpsimd.memset(blk[:], 1.0)
    nc.gpsimd.affine_select(   # keep where p - 32g >= 0
        out=blk[:], in_=blk[:], compare_op=ALU.is_ge, fill=0.0,
        base=0, pattern=[[-32, 2]], channel_multiplier=1,
    )
    nc.gpsimd.affine_select(   # keep where 31 - p + 32g >= 0  (i.e. p - 32g <= 31)
        out=blk[:], in_=blk[:], compare_op=ALU.is_ge, fill=0.0,
        base=31, pattern=[[32, 2]], channel_multiplier=-1,
    )
    # selector [2, 64]: sel[g, x] = 1 iff x//32 == g
    sel = consts.tile([2, 64], F32)
    nc.gpsimd.memset(sel[:], 0.0)
    nc.gpsimd.affine_select(   # iota = g - (x//32); equal -> fill... keep where != 0
        out=sel.rearrange("p (g x) -> p g x", g=2),
        in_=sel.rearrange("p (g x) -> p g x", g=2),
        compare_op=ALU.not_equal, fill=1.0,
        base=0, pattern=[[-1, 2], [0, 32]], channel_multiplier=1,
    )
    # W^T chunks via PE transposes, all into ONE psum bank -> one DVE copy.
    # wt[p=e_local, i=(ec, dc), d_local]: dc-major per ec so the mb matmul's
    # rhs spans both dc chunks contiguously ([P, 256] in one matmul)
    pst_w = ps_misc.tile([P, 4, P], F32, tag="mix")
    for dc in range(2):
        for ec in range(2):
            nc.tensor.transpose(
                pst_w[:, ec * 2 + dc, :], w_sb[:, dc, ec * P:(ec + 1) * P], ident[:]
            )
    wt = setup.tile([P, 4, P], F32, tag="wt")
    nc.vector.tensor_copy(wt[:], pst_w[:])
    # q^T chunks: qt[p=e_local, ec, b]
    pst_q = ps_misc.tile([P, 2, BL], F32, tag="mix")
    for ec in range(2):
        nc.tensor.transpose(
            pst_q[:, ec, :], q_sb[:, ec * P:(ec + 1) * P], ident[0:BL, 0:BL]
        )
    qt = setup.tile([P, 2, BL], F32, tag="qt")
    nc.vector.tensor_copy(qt[:], pst_q[:])

    # mb[p', d] = mids[b, d] = sum_e W[d, e] q[b, e], computed directly on
    # the PE for all 128 partitions at once: lhsT = qt column b broadcast
    # across the out-partition dim, rhs = the matching W^T chunk. The scans
    # read it straight from PSUM. Pair g+1's pair of mb tiles is emitted
    # right after pair g's scans (before pair g's epilogue PE work) so the
    # next pair's first scan never stalls behind the epilogue.
    ps_mb = ctx.enter_context(tc.tile_pool(name="ps_mb", bufs=2, space="PSUM"))

    def emit_mb(g):
        mb = ps_mb.tile([P, 2, D], F32)
        for b_local in range(2):
            b = g * 2 + b_local
            for ec in range(2):
                nc.tensor.matmul(
                    mb[:, b_local, :],
                    lhsT=qt[:, ec, b:b + 1].broadcast_to([P, P]),
                    rhs=wt[:, ec * 2:(ec + 1) * 2, :],
                    start=(ec == 0), stop=(ec == 1),
                )
        return mb

    mb_ps = emit_mb(0)

    # ---------------- Phase 1: main loop + epilogue per batch-pair ----------------
    for g in range(BL // 2):                 # 4 pairs
        scores = scpool.tile([P, 64], F32)   # col = b_local*32 + h*16 + s
        # prefetch this pair's mask tile (needed only in the epilogue)
        mt = mtpool.tile([64, P], F32, tag="mt")
        nc.gpsimd.dma_start(
            out=mt[:],
            in_=mp.ap()[g * 2:(g + 1) * 2].rearrange("b c p -> (b c) p"),
        )

        for b_local in range(2):
            b = g * 2 + b_local
            for h in range(2):
                c0 = b_local * 32 + h * 16
                # the very first and very last macros are split into two
                # half-tiles: the first so scan 0 starts ~4us earlier (smaller
                # first transfer), the last so the final scan after the stream
                # ends is 2.2us instead of 4.4us
                split = (g == 0 and b_local == 0 and h == 0) or (
                    g == 3 and b_local == 1 and h == 1
                )
                if split:
                    src = k.ap()[b, h * 2048:(h + 1) * 2048, :].rearrange(
                        "(p half tt) d -> half p tt d", p=P, half=2
                    )
                    subs = [(kpool_s, cumpool_s, TT // 2, src[0]),
                            (kpool_s, cumpool_s, TT // 2, src[1])]
                else:
                    subs = [(kpool, cumpool, TT,
                             k.ap()[b, h * 2048:(h + 1) * 2048, :].rearrange(
                                 "(p tt) d -> p tt d", p=P))]
                for si, (kp, cp, tts, src_ap) in enumerate(subs):
                    kt = kp.tile([P, tts, D], F32, tag="ktile")
                    nc.sync.dma_start(out=kt[:], in_=src_ap)
                    # one streaming pass: cum[p, s, j] = cumsum over the flat
                    # per-partition stream of kt[p]*mids[b]
                    cum = cp.tile([P, tts, D], F32, tag="cum")
                    nc.vector._custom_dve(
                        PRODUCT_CUMSUM,
                        out=cum[:],
                        in0=kt[:],
                        in1=mb_ps[:, b_local, :].unsqueeze(1).broadcast_to(
                            [P, tts, D]
                        ),
                    )
                    # page-end differences -> the dot products of this tile
                    cc = c0 + si * tts
                    ce = cum[:, :, D - 1:D].rearrange("p s o -> p (s o)")
                    nc.vector.tensor_copy(scores[:, cc:cc + 1], ce[:, 0:1])
                    nc.vector.tensor_tensor(
                        out=scores[:, cc + 1:cc + tts],
                        in0=ce[:, 1:tts],
                        in1=ce[:, 0:tts - 1],
                        op=ALU.subtract,
                    )

        # queue the next pair's mb matmuls ahead of this pair's epilogue PE ops
        mb_next = emit_mb(g + 1) if g + 1 < BL // 2 else None

        # ---- epilogue for this pair of batches ----
        th = epool.tile([P, 64], F32, tag="th")
        nc.scalar.activation(out=th[:], in_=scores[:], func=AF.Tanh,
                             bias=bias_col[:], scale=1.0)
        ex = epool.tile([P, 64], F32, tag="ex")
        nc.scalar.activation(out=ex[:], in_=th[:], func=AF.Exp)
        pse = ps_e.tile([64, P], F32)
        nc.tensor.transpose(pse[:], ex[:], ident[:])

        ee = epool.tile([64, P], F32, tag="ee")
        rs = epool.tile([64, 1], F32, tag="rs")
        nc.vector.scalar_tensor_tensor(
            out=ee[:], in0=pse[:], scalar=0.0, in1=mt[:],
            op0=ALU.bypass, op1=ALU.mult, accum_out=rs[:],
        )
        pss = ps_misc.tile([2, 1], F32, tag="mix")
        nc.tensor.matmul(pss[:], lhsT=blk[:], rhs=rs[:], start=True, stop=True)
        rc = epool.tile([2, 1], F32, tag="rc")
        nc.vector.reciprocal(rc[:], pss[:])
        psr2 = ps_misc.tile([64, 1], F32, tag="mix")
        nc.tensor.matmul(psr2[:], lhsT=sel[:], rhs=rc[:], start=True, stop=True)
        rcol = epool.tile([64, 1], F32, tag="rcol")
        nc.vector.tensor_copy(rcol[:], psr2[:])
        attn = epool.tile([64, P], F32, tag="attn")
        nc.scalar.activation(out=attn[:], in_=ee[:], func=AF.Copy, scale=rcol[:])
        nc.gpsimd.dma_start(
            out=out.ap()[g * 2:(g + 1) * 2].rearrange("b c p -> (b c) p"),
            in_=attn[:],
        )
        mb_ps = mb_next


def _install_ntff_hook_shim():
    """Provide antenv.axon_hooks via ctypes into libaxon_pjrt.so (the agent
    image's antenv stub lacks it), enabling NTFF capture under trace=True."""
    import sys
    import types
    import ctypes
    import contextlib

    if "antenv.axon_hooks" in sys.modules:
        return
    so = "/opt/axon/libaxon_pjrt.so"
    if not os.path.exists(so):
        return
    lib = ctypes.CDLL(so)
    if not hasattr(lib, "axon_start_nrt_profile"):
        return
    lib.axon_start_nrt_profile.argtypes = [
        ctypes.POINTER(ctypes.c_int64), ctypes.c_size_t,
    ]
    lib.axon_start_nrt_profile.restype = ctypes.c_int64
    lib.axon_stop_nrt_profile.argtypes = [ctypes.c_char_p]
    lib.axon_stop_nrt_profile.restype = ctypes.c_int64

    @contextlib.contextmanager
    def _hook(output_dir, device_ids):
        import jax

        jax.devices()
        if device_ids:
            ids = (ctypes.c_int64 * len(device_ids))(*device_ids)
            rc = lib.axon_start_nrt_profile(ids, len(device_ids))
        else:
            rc = lib.axon_start_nrt_profile(None, 0)
        if rc != 0:
            raise RuntimeError(f"axon_start_nrt_profile rc={rc}")
        try:
            yield
        finally:
            n = lib.axon_stop_nrt_profile(str(output_dir).encode())
            print(f"profile: {n} file(s) written to {output_dir}", file=sys.stderr)

    mod = types.ModuleType("antenv.axon_hooks")
    mod.get_axon_ntff_profile_hook = lambda: _hook
    mod.set_axon_ntff_profile_hook = lambda h: None
    import antenv

    sys.modules["antenv.axon_hooks"] = mod
    antenv.axon_hooks = mod


_CACHE = {}


def _get_nc():
    if "nc" not in _CACHE:
        from contextlib import ExitStack

        nc = bacc.Bacc("TRN2", debug=False)
        ins = {
            "q": nc.dram_tensor("q", [BL, D], F32, kind="ExternalInput"),
            "k": nc.dram_tensor("k", [BL, T, D], F32, kind="ExternalInput"),
            "mp": nc.dram_tensor("mp", [BL, 32, P], F32, kind="ExternalInput"),
            "W": nc.dram_tensor("W", [D, D], F32, kind="ExternalInput"),
            "bias": nc.dram_tensor("bias", [1, 1], F32, kind="ExternalInput"),
        }
        outs = {"out": nc.dram_tensor("out", [BL, 32, P], F32, kind="ExternalOutput")}
        with tile.TileContext(nc) as tc:
            with ExitStack() as ctx:
                _build_kernel(ctx, tc, outs, ins)
        nc.compile()
        _CACHE["nc"] = nc
    return _CACHE["nc"]


def kernel(q, k, m, W, bias):
    global LAST_RESULTS
    q = np.ascontiguousarray(q, dtype=np.float32)
    k = np.ascontiguousarray(k, dtype=np.float32)
    m = np.ascontiguousarray(m, dtype=np.float32)
    W = np.ascontiguousarray(W, dtype=np.float32)
    bias = np.ascontiguousarray(bias, dtype=np.float32).reshape(1, 1)

    # host-side input marshalling: permute m to the kernel's score layout.
    # mp[b, h*16+s, p] = m[b, h*2048 + p*16 + s]
    mp = np.ascontiguousarray(
        m.reshape(B, H, P, TT).transpose(0, 1, 3, 2).reshape(B, H * TT, P)
    )
    trace = bool(int(os.environ.get("KERNEL_TRACE", "0")))
    if trace:
        _install_ntff_hook_shim()
    nc = _get_nc()
    in_maps = [
        {
            "q": q[i * BL:(i + 1) * BL],
            "k": k[i * BL:(i + 1) * BL],
            "mp": mp[i * BL:(i + 1) * BL],
            "W": W,
            "bias": bias,
        }
        for i in range(NCORES)
    ]
    res = run_bass_kernel_spmd(
        nc,
        in_maps,
        core_ids=list(range(NCORES)),
        trace=trace,
    )
    LAST_RESULTS = res

    full = np.concatenate([res.results[i]["out"] for i in range(NCORES)], axis=0)
    # inverse permutation back to natural [B, T]
    out = np.ascontiguousarray(
        full.reshape(B, H, TT, P).transpose(0, 1, 3, 2).reshape(B, T)
    )
    return out
